# revision 24
# baseline (speedup 1.0000x reference)
"""Trainium2 Bass kernel for CSAM channel self-attention module.

Per batch b (one per NeuronCore, B=8 over 8 cores):
    v      = x2[b].reshape(7, D)                 # D = 64*128*128 = 1048576
    E      = v @ v.T                             # [7,7] gram ("energy")
    att    = softmax(rowmax(E) - E, axis=-1)     # == exp(rowmin(E)-E)/Z
    out    = att @ v
    y[b]   = x1[b] * (gamma*out) + x1[b] = x1[b] * (gamma*out + 1)

The host casts x1/x2 to fp16 and upcasts y from fp16, so device HBM traffic
is 3 x 14.7MB per core.  fp16 is safe: top-2 energy gaps are >128 while the
fp16 gram error is ~+-2, so att is one-hot to ~1e-56 and the y-path error is
~9e-4 relative.

Layout: d = q*65536 + w*4096 + f  (Q=16 runs, tiles [112, 4096] with
partition p = 7*q + m and 8KB contiguous DRAM lines).  The x2 cache is ONE
SBUF mega-tile xh[113, 16*4096] fp16 whose 113th row is a single ones row
(written once by one broadcast DMA) implementing the fused "+1".

Pass A: x2 tiles stream on the two HWDGE rings (SP even w, ACT odd w); the
8 x1 prefetches are queued BEHIND the x2 tiles on the same rings so x2 gets
the full device early and the x1 tail drains during the PE tail/softmax gap.
Each [112,128] chunk is transposed by a REGULAR matmul against identity
(counts as PE-busy for the HAM clock gate; transpose-mode does not) into
PSUM f32 groups of 8 chunks, cast-copied to SBUF f16 (ACT/DVE alternating),
and gram-accumulated into E_psum[112,112] trailing by PIPE chunks.

Softmax + weight build run fully ON-CHIP (no DRAM bounces): E block-diag ->
E7 via 16 accumulating selector matmuls against identity columns on a
1/64-scaled fp16 copy of E (safe: the scale error is common-mode per row and
the top-2 gap is >>), softmax with Exp activation, then W[113,112] =
blockdiag((gamma*att)^T) + ones row via one broadcast matmul + mask mul.

Pass B: op_psum[112,1024](f32, 2 banks, 4 in flight) = W.T @ xh slices;
y = op * x1 computed IN-PLACE into the x1 tile.  Muls are split to balance
engines: per 4096-tile [DVE-direct, ACT-copy->Pool-mul, ACT-copy->DVE-fp16-
mul, DVE-direct].  After tile w finishes its matmuls, its dead xh slot is
the landing buffer for x1 tile w+8 (SP queue), so x1 and x2 are each read
from HBM exactly once.  y DMAs ride ACT (first half) / SP (second half).
"""

import sys

import numpy as np

try:
    import concourse.bass as bass
except ImportError:  # grading env fallback
    sys.path.insert(0, "/opt/trn_rl_repo")
    import concourse.bass as bass

from contextlib import ExitStack

import concourse.bacc as bacc
import concourse.tile as tile
from concourse import mybir
from concourse.bass_utils import run_bass_kernel_spmd
from concourse.masks import make_identity

F32 = mybir.dt.float32
F16 = mybir.dt.float16

B = 8
NN = 7              # attention dim
Q = 16              # d-runs per channel
P = NN * Q          # 112 partitions of (q, m)
PK = P + 1          # +1 ones row for the fused "+1"
FS = 4096           # stream tile free dim (8KB fp16 DRAM lines)
FM = 512            # matmul slice free dim (one 2KB fp32 PSUM bank)
OPW = 1024          # pass-B op tile width (2 PSUM banks, one DVE mul)
D_FULL = 64 * 128 * 128
N_CORES = 8
PIPE = 20           # gram matmul trails transposes by this many chunks
ESCALE = 1.0 / 64.0  # fp16 E copy scale (keeps |E|<=65504/4)


def build_nc(d_total=D_FULL):
    assert d_total % (Q * FS) == 0
    ws = d_total // (Q * FS)          # stream tiles (16 at full size)
    cpt = FS // 128                   # transpose chunks per stream tile (32)
    n_gram = ws * cpt
    npre = max(1, ws // 2)            # x1 tiles prefetched during pass A

    nc = bacc.Bacc("TRN2", target_bir_lowering=False, debug=False)
    x1 = nc.dram_tensor("x1", [NN, d_total], F16, kind="ExternalInput")
    x2 = nc.dram_tensor("x2", [NN, d_total], F16, kind="ExternalInput")
    gm = nc.dram_tensor("gamma", [1], F32, kind="ExternalInput")
    y = nc.dram_tensor("y", [NN, d_total], F16, kind="ExternalOutput")

    x2v = x2[:].rearrange("m (q w f) -> q m w f", q=Q, w=ws, f=FS)
    x1v = x1[:].rearrange("m (q w f) -> q m w f", q=Q, w=ws, f=FS)
    yv = y[:].rearrange("m (q w f) -> q m w f", q=Q, w=ws, f=FS)

    with tile.TileContext(nc) as tc, ExitStack() as ctx:
        consts = ctx.enter_context(tc.tile_pool(name="consts", bufs=1))
        cache = ctx.enter_context(tc.tile_pool(name="cache", bufs=1))
        x1s = ctx.enter_context(tc.tile_pool(name="x1s", bufs=1))
        tsb = ctx.enter_context(tc.tile_pool(name="tsb", bufs=4))
        opc = ctx.enter_context(tc.tile_pool(name="opc", bufs=2))
        small = ctx.enter_context(tc.tile_pool(name="small", bufs=1))
        dramp = ctx.enter_context(tc.tile_pool(name="dramp", bufs=1, space="DRAM"))
        # pass-A PSUM pools live in a nested scope so pass B gets all 8 banks
        actx = ExitStack()
        tps = actx.enter_context(tc.tile_pool(name="tps", bufs=3, space="PSUM"))
        eps = actx.enter_context(tc.tile_pool(name="eps", bufs=1, space="PSUM"))
        wps = actx.enter_context(tc.tile_pool(name="wps", bufs=1, space="PSUM"))

        ident = consts.tile([P, P], F16)
        make_identity(nc, ident)
        ones = consts.tile([1, 512], F16)
        nc.vector.memset(ones[:], 1.0)

        # block-diag mask (1 on the 16 [7,7] q-blocks) and the q-tiling
        # selector T7t[j, 7q+m] = (j == m), both built once off critical path.
        # Compute engines can't start at unaligned partitions, so the mask's
        # q-blocks are scattered via a DRAM bounce at startup.
        mask = consts.tile([P, P], F16)
        nc.vector.memset(mask[:], 0.0)
        maskscr = dramp.tile([P, P], F16)
        nc.gpsimd.dma_start(out=maskscr[:], in_=mask[:])
        mask_diag = bass.AP(                            # dims (m, q, n)
            tensor=maskscr.tensor, offset=maskscr.offset,
            ap=[[P, NN], [NN * P + NN, Q], [1, NN]],
        )
        tmp7 = consts.tile([NN, P], F16)
        nc.vector.memset(tmp7[:], 1.0)
        ones_blk = bass.AP(                             # [m, q, n] of ones
            tensor=tmp7.tensor, offset=tmp7.offset,
            ap=[[tmp7.ap[0][0], NN], [NN, Q], [1, NN]],
        )
        nc.gpsimd.dma_start(out=mask_diag, in_=ones_blk)
        nc.gpsimd.dma_start(out=mask[:], in_=maskscr[:])
        t7t = consts.tile([NN, P], F16)
        nc.vector.memset(t7t[:], 0.0)
        for q in range(Q):
            s = slice(NN * q, NN * q + NN)
            nc.vector.tensor_copy(out=t7t[0:NN, s], in_=ident[0:NN, 0:NN])

        # preload the Exp activation table so the softmax chain doesn't pay a
        # mid-kernel table load
        warm_ex = small.tile([1, 1], F32)
        nc.scalar.activation(out=warm_ex[:], in_=ones[0:1, 0:1],
                             func=mybir.ActivationFunctionType.Exp)

        # persistent W tile; ones row set once here, data rows written in the
        # softmax gap
        wt = small.tile([PK, P], F16)
        nc.gpsimd.dma_start(out=wt[P:PK, :], in_=ones[0:1, 0:P])

        a32 = small.tile([32, 32], F32)
        nc.vector.memset(a32[:], 0.0)

        gsb = small.tile([NN, 1], F32)
        nc.gpsimd.dma_start(
            out=gsb[:],
            in_=bass.AP(tensor=gm[:].tensor, offset=0, ap=[[0, NN], [1, 1]]),
        )

        # x2 cache: one mega-tile (range-level dep tracking is precise) so
        # the ones row is a single broadcast DMA and x2 transfers can be
        # batched into multi-tile DMAs for single-ring efficiency
        xh = cache.tile([PK, ws * FS], F16, name="xh", tag="xh")
        ones_row = bass.AP(
            tensor=ones.tensor, offset=ones.offset,
            ap=[list(ones.ap[0]), [0, ws * FS // 512], [1, 512]],
        )
        nc.gpsimd.dma_start(out=xh[P:PK, :], in_=ones_row)

        x1c = x1s.tile([P, npre * FS], F16, name="x1c", tag="x1c")

        GRP = 8                       # transpose chunks batched per PSUM tile
        E = eps.tile([P, P], F32)

        # ~4us of dummy matmuls so the PE HAM clock-gate opens before the
        # real pass-A stream arrives (and stays open)
        for _ in range(40):
            wm = tps.tile([128, GRP * 128], F32, tag="tp")
            nc.tensor.matmul(wm[0:P, 0:P], lhsT=ident[:], rhs=ident[:],
                             start=True, stop=True)

        # ---------------- pass A: stream x2, transpose, gram ---------------
        # Transposes are REGULAR matmuls against identity (out = chunk^T @ I):
        # same cycles as transpose-mode but they count as PE-busy for the HAM
        # clock gate (transpose-mode does not and strands the PE at 1.2 GHz).
        pend = []          # tt slices awaiting gram matmul
        gi = 0             # gram matmuls emitted

        def emit_gram(tt_ap):
            nonlocal gi
            nc.tensor.matmul(E[:], lhsT=tt_ap, rhs=tt_ap,
                             start=(gi == 0), stop=(gi == n_gram - 1))
            gi += 1

        # All pass-A DMA issues are hoisted ahead of the compute loop: the
        # engine FIFOs would otherwise chain later x2 issues behind cast
        # copies that wait on the PE, serializing transfers with compute.
        # x2 rides SP (even w) / ACT (odd w); the x1 prefetches are queued
        # behind all x2 on the same rings, so per-ring FIFO order gives x2
        # the full device early and drains the x1 tail during the PE tail
        # and softmax gap.
        # x2 rides the SP (sync) queue: its engine runs no compute, so the
        # per-DMA completion-semaphore waits never block compute behind them
        # in an engine FIFO (they stalled ACT for ~50us when x2 rode
        # nc.scalar).  One HWDGE ring fans each transfer across all 16 SDMA
        # engines; transfers are batched (halves -> single -> pair -> quads)
        # to amortize the ~0.9us inter-transfer gap while still delivering
        # early tiles ahead of the PE.
        x2f = x2[:].rearrange("m (q f) -> q m f", q=Q)  # f = ws*FS per q-run
        batches = [(0, FS // 2), (FS // 2, FS)]
        pos = FS
        for nb in (1, 2, 4, 4, 4):
            if pos >= ws * FS:
                break
            end = min(pos + nb * FS, ws * FS)
            batches.append((pos, end))
            pos = end
        while pos < ws * FS:
            end = min(pos + 4 * FS, ws * FS)
            batches.append((pos, end))
            pos = end
        for lo, hi in batches:
            nc.sync.dma_start(out=xh[0:P, lo:hi], in_=x2f[:, :, lo:hi])
        # x1 prefetches ride SP behind all of x2 (per-ring FIFO = x2 gets the
        # full device early, the x1 tail drains during the PE tail and gap),
        # batched in pairs so the first pair lands before pass-B tile 0
        x1f = x1[:].rearrange("m (q f) -> q m f", q=Q)
        for k in range(0, npre, 2):
            hi = min(k + 2, npre) * FS
            nc.sync.dma_start(out=x1c[0:P, k * FS:hi],
                              in_=x1f[:, :, k * FS:hi])

        for w in range(ws):
            base = w * FS
            for g in range(cpt // GRP):
                # chunks live in 128-col slots so each 112-col matmul output
                # stays inside one 2KB PSUM bank
                tp = tps.tile([128, GRP * 128], F32, tag="tp")
                for kk in range(GRP):
                    c = g * GRP + kk
                    nc.tensor.matmul(
                        tp[:, kk * 128:kk * 128 + P],
                        lhsT=xh[0:P, base + c * 128:base + (c + 1) * 128],
                        rhs=ident[:], start=True, stop=True)
                tt = tsb.tile([128, GRP * 128], F16)
                # copy only the written 112-col slice of each 128-col slot
                tp_w = bass.AP(tensor=tp.tensor, offset=tp.offset,
                               ap=[[tp.ap[0][0], 128], [128, GRP], [1, P]])
                tt_w = bass.AP(tensor=tt.tensor, offset=tt.offset,
                               ap=[[tt.ap[0][0], 128], [128, GRP], [1, P]])
                if (w * (cpt // GRP) + g) % 2 == 0:
                    nc.scalar.copy(tt_w, tp_w)
                else:
                    nc.vector.tensor_copy(out=tt_w, in_=tp_w)
                for kk in range(GRP):
                    pend.append(tt[:, kk * 128:kk * 128 + P])
                while len(pend) > PIPE:
                    emit_gram(pend.pop(0))
        for tt_ap in pend:
            emit_gram(tt_ap)
        pend = []

        # ---------------- energy -> attention -> weights (all on-chip) -----
        # fp16 copy of E at 1/64 scale: the fp16 error on the dominant diag
        # is common-mode per row (softmax-invariant); off-diag error ~+-0.5
        # vs a >128 top-2 gap
        e16 = small.tile([P, P], F16)
        nc.scalar.activation(out=e16[:], in_=E[:],
                             func=mybir.ActivationFunctionType.Copy,
                             scale=ESCALE)
        # E7[n,m] = sum_q E[7q+n, 7q+m]: 16 accumulating selector matmuls
        e7p = wps.tile([P, P], F32, name="e7p", tag="wps")
        for q in range(Q):
            s = slice(NN * q, NN * q + NN)
            nc.tensor.matmul(e7p[0:NN, 0:NN], lhsT=ident[:, s], rhs=e16[:, s],
                             start=(q == 0), stop=(q == Q - 1))
        mn = small.tile([NN, 1], F32)
        nc.vector.tensor_reduce(
            out=mn[:], in_=e7p[0:NN, 0:NN], axis=mybir.AxisListType.X,
            op=mybir.AluOpType.min,
        )
        d7 = small.tile([NN, NN], F32)
        nc.vector.tensor_scalar_sub(d7[:], e7p[0:NN, 0:NN], mn[:])
        ex = small.tile([NN, NN], F32)
        nc.scalar.activation(
            out=ex[:], in_=d7[:], func=mybir.ActivationFunctionType.Exp,
            scale=-1.0 / ESCALE,
        )                                              # exp(rowmin - E7)
        z = small.tile([NN, 1], F32)
        nc.vector.tensor_reduce(
            out=z[:], in_=ex[:], axis=mybir.AxisListType.X,
            op=mybir.AluOpType.add,
        )
        r = small.tile([NN, 1], F32)
        nc.vector.reciprocal(r[:], z[:])
        rg = small.tile([NN, 1], F32)
        nc.vector.tensor_mul(rg[:], r[:], gsb[:])      # gamma / Z_n
        nc.vector.tensor_scalar_mul(a32[0:NN, 0:NN], ex[:], rg[:])
        at32 = small.tile([32, 32], F32)
        nc.vector.transpose(at32[:], a32[:])           # (gamma*att)^T
        # atile[j, 7q+n] = at[j, n] for all q
        atile = small.tile([NN, P], F16)
        for q in range(Q):
            s = slice(NN * q, NN * q + NN)
            nc.vector.tensor_copy(out=atile[0:NN, s], in_=at32[0:NN, 0:NN])
        # W = blockdiag((gamma*att)^T): broadcast matmul + mask
        wp = wps.tile([P, P], F32, name="wp", tag="wps")
        nc.tensor.matmul(wp[:], lhsT=t7t[:], rhs=atile[:], start=True,
                         stop=True)
        nc.vector.tensor_mul(wt[0:P, :], wp[:], mask[:])

        actx.close()                                   # free pass-A PSUM
        ops = ctx.enter_context(tc.tile_pool(name="ops", bufs=4, space="PSUM"))

        # ---------------- pass B: op = W.T @ Xh; y = op * x1 (in place) -----
        # Pool (gpsimd) cannot read PSUM, so muls are split: per tile the 4
        # [112,1024] op tiles go [DVE-direct, ACT-copy->Pool, ACT-copy->
        # DVE-fp16, DVE-direct]; the last tile is all-DVE to shorten the
        # Pool tail.  y DMAs ride ACT (first half) / SP (second half),
        # in-pass x1 loads ride SP.
        mpt = FS // OPW
        for w in range(ws):
            base = w * FS
            op_t = []
            for j in range(mpt):
                op = ops.tile([P, OPW], F32, tag="op")
                for h in range(OPW // FM):
                    sl = slice(base + j * OPW + h * FM,
                               base + j * OPW + (h + 1) * FM)
                    nc.tensor.matmul(op[:, h * FM:(h + 1) * FM],
                                     lhsT=wt[:], rhs=xh[:, sl],
                                     start=True, stop=True)
                op_t.append(op)
            if w + npre < ws:
                # xh slot w is dead after its matmuls: land x1 tile w+npre
                # on the otherwise-idle Pool (SWDGE) ring so it streams
                # concurrently with the SP y stream
                nc.gpsimd.dma_start(out=xh[0:P, base:base + FS],
                                    in_=x1v[:, :, w + npre, :])
            if w < npre:
                dst = x1c[0:P, w * FS:(w + 1) * FS]
            else:
                dst = xh[0:P, (w - npre) * FS:(w - npre + 1) * FS]
            for j, op in enumerate(op_t):
                sl = slice(j * OPW, (j + 1) * OPW)
                kind = ("direct", "pool", "dve16", "direct")[j]
                if w == ws - 1:
                    kind = ("direct", "direct", "dve16", "direct")[j]
                if kind == "direct":
                    nc.vector.tensor_mul(dst[:, sl], op[:], dst[:, sl])
                else:
                    oc = opc.tile([P, OPW], F16)
                    nc.scalar.copy(oc[:], op[:])
                    if kind == "pool":
                        nc.gpsimd.tensor_mul(dst[:, sl], oc[:], dst[:, sl])
                    else:
                        nc.vector.tensor_mul(dst[:, sl], oc[:], dst[:, sl])
            nc.sync.dma_start(out=yv[:, :, w, :], in_=dst[:, :])

    nc.compile()
    return nc


_NC_CACHE = {}


def _get_nc(d_total=D_FULL):
    if d_total not in _NC_CACHE:
        _NC_CACHE[d_total] = build_nc(d_total)
    return _NC_CACHE[d_total]


def kernel(x1: np.ndarray, x2: np.ndarray, gamma: np.ndarray) -> np.ndarray:
    b, n, c, h, w = x1.shape
    assert (b, n) == (B, NN)
    d = c * h * w
    x1r = np.ascontiguousarray(x1.reshape(b, n, d)).astype(np.float16)
    x2r = np.ascontiguousarray(x2.reshape(b, n, d)).astype(np.float16)
    g = np.asarray(gamma, dtype=np.float32).reshape(1)

    nc = _get_nc(d)
    in_maps = [
        {"x1": x1r[i], "x2": x2r[i], "gamma": g} for i in range(N_CORES)
    ]
    res = run_bass_kernel_spmd(nc, in_maps, list(range(N_CORES)))
    out = np.stack([res.results[i]["y"] for i in range(N_CORES)], axis=0)
    return out.reshape(b, n, c, h, w).astype(np.float32)
